# revision 1
# baseline (speedup 1.0000x reference)
"""Trainium2 Bass kernel for nn_Loss_dict_50646254354805 (NeRF-style loss).

Self-contained: accepts FULL inputs, shards across 8 NeuronCores (rays for
the per-ray losses, samples for the hash loss), runs one SPMD Bass module,
host-sums the 8 partial scalars.

Inter-loss algorithm (per ray, per prop level): the reference's
blur_step_function + sorted_interp_quad is reproduced exactly in a "merged
domain": tag query/event values in 2 mantissa LSBs, bitonic-merge the events
(sdist-+pw) with the prop_sdist queries, rebuild the blurred-density CDF with
prefix scans (matching the reference's cumsum structure), and compact the CDF
at query slots with per-partition local_scatter. No per-ray gather needed.
"""
import numpy as np

import concourse.bass as bass
import concourse.mybir as mybir
import concourse.tile as tile
from concourse import bacc
from concourse.bass_utils import run_bass_kernel_spmd

dt = mybir.dt
Alu = mybir.AluOpType
AX = mybir.AxisListType
P = 128

# problem constants
PULSE = (0.01, 0.005)
W_RGB, W_INTER, W_DIST, W_HASH = 1.0, 1.0, 0.01, 0.1
NUM_SEGMENTS = 65536
R, N = 4096, 48
M = R * N
N_CORES = 8
RPC = R // N_CORES            # rays per core (512)
NBLK = RPC // P               # ray tiles per core (4)
MPC = M // N_CORES            # hash samples per core (24576)
HALO = 64                     # hash run halo
HROW = MPC // P               # hash samples per partition (192)
HCOLS = HROW + HALO + 1       # loaded cols per partition (257)
HSLICE = HALO + MPC + HALO    # per-core hash slice length (24704)

# per-level geometry
LVL = {0: dict(X=257, n2=512), 1: dict(X=97, n2=256)}
for _L in LVL.values():
    _L["EW"] = ((_L["X"] + 98 + 1 + 7) // 8) * 8        # 360 / 200
    _L["QW"] = _L["n2"] - 96                            # 416 / 160
BIGF = 3.0                    # merge pad value (> max real value ~2.02)


def _ts_int(eng, out, in0, imm1, op0, imm2=None, op1=None):
    """tensor_scalar with int32 immediates (for bitwise ops)."""
    ins_ = [eng.lower_ap(in0), mybir.ImmediateValue(dtype=dt.int32, value=int(imm1))]
    kw = dict(op0=op0)
    if imm2 is not None:
        ins_.append(mybir.ImmediateValue(dtype=dt.int32, value=int(imm2)))
        kw["op1"] = op1
    return eng.add_instruction(mybir.InstTensorScalarPtr(
        name=eng.bass.get_next_instruction_name(),
        ins=ins_, outs=[eng.lower_ap(out)], **kw))


def _bcast_row(nc, dst_ap, src_ap, n, eng=None):
    """DMA a replicated HBM const [P, n] into dst [P, n]."""
    eng = eng or nc.scalar
    eng.dma_start(dst_ap, src_ap[:, 0:n])


def _blk(ap, n2):
    """[P, NBLK*n2] AP -> [P, NBLK, n2] view."""
    return ap.rearrange("p (b n) -> p b n", b=NBLK)


def _bitonic_merge(eng, bufa, bufb, width, descending):
    """Ping-pong bitonic merge over [P, NBLK*width] f32-viewed int tiles.

    Returns (result_buf, scratch_buf)."""
    cur, nxt = bufa, bufb
    d = width // 2
    while d >= 1:
        c3 = cur[:].bitcast(dt.float32).rearrange("p (c td) -> p c td", td=2 * d)
        n3 = nxt[:].bitcast(dt.float32).rearrange("p (c td) -> p c td", td=2 * d)
        lo_in, hi_in = c3[:, :, 0:d], c3[:, :, d:2 * d]
        if descending:
            eng.tensor_tensor(n3[:, :, 0:d], lo_in, hi_in, Alu.max)
            eng.tensor_tensor(n3[:, :, d:2 * d], lo_in, hi_in, Alu.min)
        else:
            eng.tensor_tensor(n3[:, :, 0:d], lo_in, hi_in, Alu.min)
            eng.tensor_tensor(n3[:, :, d:2 * d], lo_in, hi_in, Alu.max)
        cur, nxt = nxt, cur
        d //= 2
    return cur, nxt


def _split_u16(nc, ap_f32_src, lo_dst, hi_dst):
    vu = ap_f32_src.bitcast(dt.uint16).rearrange("p (n two) -> p n two", two=2)
    nc.gpsimd.tensor_copy(lo_dst, vu[:, :, 0])
    nc.gpsimd.tensor_copy(hi_dst, vu[:, :, 1])


def _emit_level(nc, tc, pool, lvl, s_sh, radio, x_ap, pwt_ap, inter_acc, aps,
                VM=None, VS=None, VE=None):
    """Inter-loss pipeline for one prop level. Careful manual buffer reuse.

    Post-merge work runs in a compact [NBLK, LW] layout (LW = EW + 24) to cut
    scan/elementwise volume; the merge itself needs pow2 [NBLK, n2] blocks."""
    VM = VM or nc.vector
    VS = VS or nc.vector
    VE = VE or nc.vector
    L = LVL[lvl]
    X, n2, EW, QW = L["X"], L["n2"], L["EW"], L["QW"]
    LW = EW + 24
    SL = NBLK * n2
    NL = NBLK * LW
    NEV = NBLK * EW
    pw = PULSE[lvl]

    def blkL(ap):
        return ap.rearrange("p (b n) -> p b n", b=NBLK)

    # ---------- big slots: B0/B1 merge-sized, B2..B7 compact ----------
    B0 = pool.tile([P, SL], dt.float32, tag="big0", name="big0")
    B1 = pool.tile([P, SL], dt.float32, tag="big1", name="big1")
    bigs = []
    for i in range(2, 8):
        b = pool.tile([P, NL], dt.float32, tag=f"big{i}", name=f"big{i}")
        bigs.append(b)
    B2, B3, B4, B5, B6, B7 = bigs

    # ---------- per-level constants ----------
    iota_loc16 = pool.tile([P, SL], dt.int16, tag="iota_loc16")
    _bcast_row(nc, iota_loc16[:], aps[f"c_iota16_l{lvl}"], SL)
    iotaP1f = pool.tile([P, NL], dt.float32, tag="iotaP1f")
    _bcast_row(nc, iotaP1f[:], aps[f"c_iotap1_l{lvl}"], NL)
    mask_scan = pool.tile([P, NL], dt.float32, tag="mask_scan")
    _bcast_row(nc, mask_scan[:], aps[f"c_mask_l{lvl}"], NL)

    # ---------- inputs ----------
    xt = pool.tile([P, NBLK * X], dt.float32, tag="xt")
    nc.sync.dma_start(_blk(xt[:], X), x_ap.rearrange("(b p) x -> p b x", p=P))
    pwt = pool.tile([P, NBLK * (X - 1)], dt.float32, tag="pwt")
    nc.sync.dma_start(_blk(pwt[:], X - 1), pwt_ap.rearrange("(b p) x -> p b x", p=P))

    # ---------- exact shifted event values ----------
    emsh = pool.tile([P, NBLK * 49], dt.float32, tag="emsh")
    nc.scalar.activation(emsh[:], s_sh[:],
                         mybir.ActivationFunctionType.Copy, bias=1.0 - pw)
    epsh = pool.tile([P, NBLK * 49], dt.float32, tag="epsh")
    nc.scalar.activation(epsh[:], s_sh[:],
                         mybir.ActivationFunctionType.Copy, bias=1.0 + pw)

    # ---------- B1 merge: tagged events, descending ----------
    b1a = pool.tile([P, NBLK * 128], dt.int32, tag="b1a")
    b1b = pool.tile([P, NBLK * 128], dt.int32, tag="b1b")
    _bcast_row(nc, b1a[:].bitcast(dt.float32), aps["c_bigf"], NBLK * 128)
    b1a3 = _blk(b1a[:], 128)
    _ts_int(nc.vector, b1a3[:, :, 0:49], _blk(emsh[:], 49).bitcast(dt.int32),
            ~3, Alu.bitwise_and, 1, Alu.bitwise_or)
    ept = pool.tile([P, NBLK * 49], dt.int32, tag="ept")
    _ts_int(nc.vector, ept[:], epsh[:].bitcast(dt.int32), ~3,
            Alu.bitwise_and, 3, Alu.bitwise_or)
    nc.vector.tensor_copy(b1a3[:, :, 79:128].bitcast(dt.float32),
                          _blk(ept[:], 49).bitcast(dt.float32)[:, :, ::-1])
    b1, _ = _bitonic_merge(VM, b1a, b1b, 128, descending=True)

    # ---------- B2 merge: queries + events, ascending ----------
    b2a = B0[:].bitcast(dt.int32)
    _bcast_row(nc, B0[:], aps["c_bigf"], SL)
    xsh = pool.tile([P, NBLK * X], dt.float32, tag="xsh")
    nc.scalar.activation(xsh[:], xt[:],
                         mybir.ActivationFunctionType.Copy, bias=1.0)
    b2a3 = _blk(b2a, n2)
    _ts_int(nc.vector, b2a3[:, :, 0:X], _blk(xsh[:], X).bitcast(dt.int32), ~3,
            Alu.bitwise_and)
    nc.gpsimd.tensor_copy(b2a3[:, :, n2 - 128:n2].bitcast(dt.float32),
                          _blk(b1[:], 128).bitcast(dt.float32))
    SMt, SAt = _bitonic_merge(VM, B0, B1, n2, descending=False)
    m = SMt[:].bitcast(dt.int32)       # merged tagged values, [NBLK, n2] layout
    mS = _blk(m, n2)[:, :, 0:LW]       # strided view of the real+pad prefix
    SA = SAt                           # free merge-sized big

    # ---------- tags (into compact layout) ----------
    tag = B2[:].bitcast(dt.int32)
    _ts_int(nc.vector, tag, mS, 3, Alu.bitwise_and)
    em_f = B3
    _ts_int(nc.vector, em_f[:], tag, 1, Alu.is_equal)
    ep_f = B4
    _ts_int(nc.vector, ep_f[:], tag, 3, Alu.is_equal)
    ev_f = B5
    nc.vector.tensor_tensor(ev_f[:], em_f[:], ep_f[:], Alu.add)

    # ---------- counts ----------
    C = B2                             # overwrites tag (dead)
    VS.tensor_tensor_scan(C[:], mask_scan[:], ev_f[:], 0.0, Alu.mult, Alu.add)
    Cm = B6
    VS.tensor_tensor_scan(Cm[:], mask_scan[:], em_f[:], 0.0, Alu.mult, Alu.add)
    tmpf = B7

    # ---------- event position compaction (block-local slots) ----------
    idx16 = pool.tile([P, NL], dt.int16, tag="idx16")
    pos_m = pool.tile([P, NBLK * 64], dt.int16, tag="pos_m")
    pos_p = pool.tile([P, NBLK * 64], dt.int16, tag="pos_p")
    tmpf3 = blkL(tmpf[:])
    idx163 = blkL(idx16[:])
    C3 = blkL(C[:])
    Cm3 = blkL(Cm[:])
    em3 = blkL(em_f[:])
    ep3 = blkL(ep_f[:])
    for which, pos in ((0, pos_m), (1, pos_p)):
        if which == 0:
            VE.tensor_tensor(tmpf3[:, :, 0:EW], Cm3[:, :, 0:EW],
                             em3[:, :, 0:EW], Alu.mult)
        else:
            VE.tensor_tensor(tmpf3[:, :, 0:EW], C3[:, :, 0:EW],
                             Cm3[:, :, 0:EW], Alu.subtract)
            VE.tensor_tensor(tmpf3[:, :, 0:EW], tmpf3[:, :, 0:EW],
                             ep3[:, :, 0:EW], Alu.mult)
        nc.scalar.activation(idx163[:, :, 0:EW], tmpf3[:, :, 0:EW],
                             mybir.ActivationFunctionType.Copy, bias=-1.0)
        for b in range(NBLK):
            nc.gpsimd.local_scatter(pos[:, b * 64:(b + 1) * 64],
                                    iota_loc16[:, b * n2:b * n2 + EW],
                                    idx16[:, b * LW:b * LW + EW], channels=P,
                                    num_elems=64, num_idxs=EW)

    # ---------- radio + exact event value scatters (targets in LW coords) ----
    tgt16 = pool.tile([P, NBLK * 128], dt.int16, tag="tgt16")
    t3 = _blk(tgt16[:], 128)
    for b in range(NBLK):
        _ts_int(nc.vector, t3[:, b, 0:49], pos_m[:, b * 64:b * 64 + 49],
                b * LW, Alu.add)
        _ts_int(nc.vector, t3[:, b, 49:98], pos_p[:, b * 64:b * 64 + 49],
                b * LW, Alu.add)
    nc.gpsimd.memset(t3[:, :, 98:128], -1)

    radcat = pool.tile([P, NBLK * 128], dt.float32, tag="radcat")
    nc.gpsimd.memset(_blk(radcat[:], 128)[:, :, 98:128], 0.0)
    r3 = _blk(radcat[:], 128)
    nc.vector.tensor_copy(r3[:, :, 0:49], _blk(radio[:], 49))
    nc.vector.tensor_scalar(r3[:, :, 49:98], _blk(radio[:], 49), -1.0, None,
                            Alu.mult)
    evcat = pool.tile([P, NBLK * 128], dt.float32, tag="evcat")
    nc.gpsimd.memset(_blk(evcat[:], 128)[:, :, 98:128], 0.0)
    e3 = _blk(evcat[:], 128)
    nc.vector.tensor_copy(e3[:, :, 0:49], _blk(emsh[:], 49))
    nc.vector.tensor_copy(e3[:, :, 49:98], _blk(epsh[:], 49))

    lo_s = pool.tile([P, NBLK * 128], dt.uint16, tag="lo_s")
    hi_s = pool.tile([P, NBLK * 128], dt.uint16, tag="hi_s")
    b7u = B7[:].bitcast(dt.uint16)
    rad_lo = b7u[:, 0:NL]
    rad_hi = b7u[:, NL:2 * NL]
    vev_lo_t = pool.tile([P, NL], dt.uint16, tag="vev_lo")
    vev_hi_t = pool.tile([P, NL], dt.uint16, tag="vev_hi")
    _split_u16(nc, radcat[:], lo_s[:], hi_s[:])
    nc.gpsimd.local_scatter(rad_lo, lo_s[:], tgt16[:], channels=P,
                            num_elems=NL, num_idxs=NBLK * 128)
    nc.gpsimd.local_scatter(rad_hi, hi_s[:], tgt16[:], channels=P,
                            num_elems=NL, num_idxs=NBLK * 128)
    _split_u16(nc, evcat[:], lo_s[:], hi_s[:])
    nc.gpsimd.local_scatter(vev_lo_t[:], lo_s[:], tgt16[:], channels=P,
                            num_elems=NL, num_idxs=NBLK * 128)
    nc.gpsimd.local_scatter(vev_hi_t[:], hi_s[:], tgt16[:], channels=P,
                            num_elems=NL, num_idxs=NBLK * 128)

    # ---------- recombine radio into compact layout (ls pre-zeroed dests) ----
    radio_m = SA[:][:, 0:NL]
    rm_u = radio_m.bitcast(dt.uint16).rearrange("p (n two) -> p n two", two=2)
    nc.gpsimd.tensor_copy(rm_u[:, :, 0], rad_lo)
    nc.gpsimd.tensor_copy(rm_u[:, :, 1], rad_hi)

    # ---------- slope scan ----------
    g = B4                             # ep_f dead
    VS.tensor_tensor_scan(g[:], mask_scan[:], radio_m, 0.0, Alu.mult, Alu.add)

    # ---------- v: cleaned values, event slots replaced by exact values ------
    v = B3                             # em_f dead
    _ts_int(nc.vector, v[:].bitcast(dt.int32), mS, ~3, Alu.bitwise_and)
    vev32 = SA[:][:, 0:NL]             # radio_m dead (after g scan)
    vv_u = vev32.bitcast(dt.uint16).rearrange("p (n two) -> p n two", two=2)
    nc.gpsimd.tensor_copy(vv_u[:, :, 0], vev_lo_t[:])
    nc.gpsimd.tensor_copy(vv_u[:, :, 1], vev_hi_t[:])
    one_m_ev = B7                      # rad halves consumed by recombine above
    nc.scalar.activation(one_m_ev[:], ev_f[:],
                         mybir.ActivationFunctionType.Copy, bias=1.0, scale=-1.0)
    vf3 = blkL(v[:])
    om3 = blkL(one_m_ev[:])
    VE.tensor_tensor(vf3[:, :, 0:EW], vf3[:, :, 0:EW], om3[:, :, 0:EW], Alu.mult)
    VE.tensor_tensor(vf3[:, :, 0:EW], vf3[:, :, 0:EW],
                     blkL(vev32)[:, :, 0:EW], Alu.add)

    # ---------- density reconstruction ----------
    dv = B6                            # Cm dead
    dv3 = blkL(dv[:])
    nc.gpsimd.memset(dv3[:, :, 0:1], 0.0)
    VE.tensor_tensor(dv3[:, :, 1:EW], vf3[:, :, 1:EW], vf3[:, :, 0:EW - 1],
                     Alu.subtract)
    wg = SA[:][:, 0:NL]                # vev32 dead (after v combine)
    wg3 = blkL(wg)
    nc.gpsimd.memset(wg3[:, :, 0:1], 0.0)
    nc.gpsimd.memset(wg3[:, :, EW:LW], 0.0)
    VE.tensor_tensor(wg3[:, :, 1:EW], dv3[:, :, 1:EW],
                     blkL(g[:])[:, :, 0:EW - 1], Alu.mult)
    w = SMt                            # m dead (after v extraction)
    wv = w[:][:, 0:NL]
    VS.tensor_tensor_scan(wv, mask_scan[:], wg, 0.0, Alu.mult, Alu.add)
    wc = B3                            # v dead (after dv)
    nc.scalar.activation(wc[:], wv, mybir.ActivationFunctionType.Relu)
    scr = SA[:][:, 0:NL]               # wg dead (after w scan)
    scr3 = blkL(scr)
    wc3 = blkL(wc[:])
    VE.tensor_tensor(scr3[:, :, 1:EW], wc3[:, :, 1:EW], wc3[:, :, 0:EW - 1],
                     Alu.add)
    area = B4                          # g dead (after wg)
    a3 = blkL(area[:])
    nc.gpsimd.memset(a3[:, :, 0:1], 0.0)
    nc.gpsimd.memset(a3[:, :, EW:LW], 0.0)
    VE.scalar_tensor_tensor(a3[:, :, 1:EW], scr3[:, :, 1:EW], 0.5,
                            dv3[:, :, 1:EW], Alu.mult, Alu.mult)
    cdf = B6                           # dv dead (after area)
    VS.tensor_tensor_scan(cdf[:], mask_scan[:], area[:], 0.0, Alu.mult, Alu.add)

    # ---------- compact cdf at query slots ----------
    qf = SA[:][:, 0:NL]                # scr dead (after area)
    nc.scalar.activation(qf, ev_f[:], mybir.ActivationFunctionType.Copy,
                         bias=1.0, scale=-1.0)
    tmpf = B7                          # one_m_ev value no longer needed
    tmpf3 = blkL(tmpf[:])
    iq3 = blkL(iotaP1f[:])
    qf3 = blkL(qf)
    VE.tensor_tensor(tmpf3[:, :, 0:EW], iq3[:, :, 0:EW], C3[:, :, 0:EW],
                     Alu.subtract)
    VE.tensor_tensor(tmpf3[:, :, 0:EW], tmpf3[:, :, 0:EW], qf3[:, :, 0:EW],
                     Alu.mult)
    nc.scalar.activation(idx163[:, :, 0:EW], tmpf3[:, :, 0:EW],
                         mybir.ActivationFunctionType.Copy, bias=-1.0)
    b5u = B5[:].bitcast(dt.uint16)     # ev_f dead (after qf)
    cdf_lo16 = b5u[:, 0:NL]
    cdf_hi16 = b5u[:, NL:2 * NL]
    cdf_u = cdf[:].bitcast(dt.uint16).rearrange("p (n two) -> p n two", two=2)
    nc.gpsimd.tensor_copy(cdf_lo16, cdf_u[:, :, 0])
    nc.gpsimd.tensor_copy(cdf_hi16, cdf_u[:, :, 1])
    QWS = EW - 98                      # compact dest width (covers pad slots)
    smu = SMt[:].bitcast(dt.uint16)    # w dead (after wc)
    cq_lo = smu[:, 0:NBLK * QWS]
    cq_hi = smu[:, SL:SL + NBLK * QWS]
    for b in range(NBLK):
        nc.gpsimd.local_scatter(cq_lo[:, b * QWS:(b + 1) * QWS],
                                cdf_lo16[:, b * LW:b * LW + EW],
                                idx16[:, b * LW:b * LW + EW], channels=P,
                                num_elems=QWS, num_idxs=EW)
        nc.gpsimd.local_scatter(cq_hi[:, b * QWS:(b + 1) * QWS],
                                cdf_hi16[:, b * LW:b * LW + EW],
                                idx16[:, b * LW:b * LW + EW], channels=P,
                                num_elems=QWS, num_idxs=EW)
    cdfq = B3[:].bitcast(dt.int32)     # wc dead (after scr)
    cq_u = cdfq.bitcast(dt.uint16).rearrange("p (b n two) -> p b n two",
                                             b=NBLK, two=2)
    nc.gpsimd.tensor_copy(cq_u[:, :, 0:X, 0], _blk(cq_lo, QWS)[:, :, 0:X])
    nc.gpsimd.tensor_copy(cq_u[:, :, 0:X, 1], _blk(cq_hi, QWS)[:, :, 0:X])

    # ---------- loss tail ----------
    b4f = B4                           # area dead (after cdf scan)
    NW = NBLK * (X - 1)
    ws = b4f[:][:, 0:NW]
    cqf = cdfq.bitcast(dt.float32).rearrange("p (b n) -> p b n", b=NBLK)
    ws3 = ws.rearrange("p (b n) -> p b n", b=NBLK)
    VE.tensor_tensor(ws3, cqf[:, :, 1:X], cqf[:, :, 0:X - 1], Alu.subtract)
    VE.tensor_tensor(ws, ws, pwt[:], Alu.subtract)
    den = pool.tile([P, NW], dt.float32, tag="dent")
    nc.scalar.activation(den[:], pwt[:], mybir.ActivationFunctionType.Copy,
                         bias=1e-5)
    nc.vector.reciprocal(den[:], den[:])
    rsl = pool.tile([P, NW], dt.float32, tag="rsl")
    nc.scalar.activation(rsl[:], ws, mybir.ActivationFunctionType.Relu)
    VE.tensor_tensor(ws, ws, rsl[:], Alu.mult)
    VE.tensor_tensor(ws, ws, den[:], Alu.mult)
    part = pool.tile([P, 1], dt.float32, tag="part")
    nc.vector.tensor_reduce(part[:], ws3, AX.XY, Alu.add)
    nc.vector.tensor_scalar(inter_acc[:], part[:], 1.0 / (R * (X - 1)), None,
                            Alu.mult)


def build_module(parts=("rgb", "dist", "hash", "l0", "l1")):
    nc = bacc.Bacc("TRN2", target_bir_lowering=False, debug=False,
                   enable_asserts=False, num_devices=N_CORES)
    aps = {}

    def din(name, shape, dtype=dt.float32):
        aps[name] = nc.dram_tensor(name, shape, dtype, kind="ExternalInput").ap()
    din("pd", [RPC, 3]); din("gt", [RPC, 3])
    din("sd", [RPC, 49]); din("rw", [RPC, 48])
    din("ps0", [RPC, 257]); din("pw0", [RPC, 256])
    din("ps1", [RPC, 97]); din("pw1", [RPC, 96])
    din("hi0", [HSLICE], dt.int32); din("he0", [HSLICE * 2])
    din("hi1", [HSLICE], dt.int32); din("he1", [HSLICE * 2])
    din("c_iota16_l0", [P, NBLK * 512], dt.int16)
    din("c_iota16_l1", [P, NBLK * 256], dt.int16)
    din("c_iotap1_l0", [P, NBLK * 384]); din("c_iotap1_l1", [P, NBLK * 224])
    din("c_mask_l0", [P, NBLK * 384]); din("c_mask_l1", [P, NBLK * 224])
    din("c_mask48", [P, NBLK * 48]); din("c_ones", [P, HCOLS])
    din("c_zeros", [P, NBLK * 512]); din("c_bigf", [P, NBLK * 512])
    out_ap = nc.dram_tensor("out", [1, 1], dt.float32, kind="ExternalOutput").ap()

    with tile.TileContext(nc) as tc:
        _emit(nc, tc, aps, out_ap, parts)
    nc.compile()
    return nc


def _emit(nc, tc, aps, out_ap, parts=("rgb", "dist", "hash", "l0", "l1")):
    import contextlib
    with contextlib.ExitStack() as ctx:
        cpool = ctx.enter_context(tc.tile_pool(name="consts", bufs=1))
        mask48 = cpool.tile([P, NBLK * 48], dt.float32, tag="mask48")
        _bcast_row(nc, mask48[:], aps["c_mask48"], NBLK * 48)
        ones_h = cpool.tile([P, HCOLS], dt.float32, tag="ones_h")
        _bcast_row(nc, ones_h[:], aps["c_ones"], HCOLS)

        accs = {}
        for name in ("rgb", "inter", "inter1", "p1", "p2", "hash"):
            a = cpool.tile([P, 1], dt.float32, tag=f"acc_{name}")
            accs[name] = a

        for a in accs.values():
            nc.vector.memset(a[:], 0.0)

        # ---------- shared render tables + radio + dist ----------
        spool = ctx.enter_context(tc.tile_pool(name="shared", bufs=1))
        s_sh = spool.tile([P, NBLK * 49], dt.float32, tag="s_sh")
        nc.sync.dma_start(_blk(s_sh[:], 49),
                          aps["sd"].rearrange("(b p) x -> p b x", p=P))
        radios = {0: spool.tile([P, NBLK * 49], dt.float32, tag="radio0",
                                name="radio0"),
                  1: spool.tile([P, NBLK * 49], dt.float32, tag="radio1",
                                name="radio1")}

        with tc.tile_pool(name="setup", bufs=1) as pool:
            rw_sh = pool.tile([P, NBLK * 48], dt.float32, tag="rw_sh")
            nc.sync.dma_start(_blk(rw_sh[:], 48),
                              aps["rw"].rearrange("(b p) x -> p b x", p=P))
            s3 = _blk(s_sh[:], 49)
            ds = pool.tile([P, NBLK * 48], dt.float32, tag="ds")
            nc.vector.tensor_tensor(_blk(ds[:], 48), s3[:, :, 1:49],
                                    s3[:, :, 0:48], Alu.subtract)
            dse = pool.tile([P, NBLK * 48], dt.float32, tag="dse")
            nc.vector.tensor_scalar(dse[:], ds[:], 1e-8, None, Alu.add)
            wnorm = pool.tile([P, NBLK * 48], dt.float32, tag="wnorm")
            nc.vector.reciprocal(dse[:], dse[:])
            nc.vector.tensor_tensor(wnorm[:], rw_sh[:], dse[:], Alu.mult)
            wnp = pool.tile([P, NBLK * 50], dt.float32, tag="wnp")
            nc.vector.memset(wnp[:], 0.0)
            nc.vector.tensor_copy(_blk(wnp[:], 50)[:, :, 1:49], _blk(wnorm[:], 48))
            diff = pool.tile([P, NBLK * 49], dt.float32, tag="diff")
            wnp3 = _blk(wnp[:], 50)
            nc.vector.tensor_tensor(_blk(diff[:], 49), wnp3[:, :, 1:50],
                                    wnp3[:, :, 0:49], Alu.subtract)
            for lvl in (0, 1):
                nc.vector.tensor_scalar(radios[lvl][:], diff[:],
                                        1.0 / (2 * PULSE[lvl]), None, Alu.mult)

            # distortion
            mid = pool.tile([P, NBLK * 48], dt.float32, tag="mid")
            nc.vector.tensor_tensor(_blk(mid[:], 48), s3[:, :, 1:49],
                                    s3[:, :, 0:48], Alu.add)
            nc.vector.tensor_scalar(mid[:], mid[:], 0.5, None, Alu.mult)
            wm = pool.tile([P, NBLK * 48], dt.float32, tag="wm")
            nc.vector.tensor_tensor(wm[:], rw_sh[:], mid[:], Alu.mult)
            Cin = pool.tile([P, NBLK * 48], dt.float32, tag="Cin")
            nc.vector.tensor_tensor_scan(Cin[:], mask48[:], rw_sh[:], 0.0,
                                         Alu.mult, Alu.add)
            Sin = pool.tile([P, NBLK * 48], dt.float32, tag="Sin")
            nc.vector.tensor_tensor_scan(Sin[:], mask48[:], wm[:], 0.0,
                                         Alu.mult, Alu.add)
            A = pool.tile([P, NBLK * 47], dt.float32, tag="A47")
            m3 = _blk(mid[:], 48)
            c3 = _blk(Cin[:], 48)
            sw3 = _blk(Sin[:], 48)
            rw3 = _blk(rw_sh[:], 48)
            A3 = _blk(A[:], 47)
            nc.vector.tensor_tensor(A3, m3[:, :, 1:48], c3[:, :, 0:47], Alu.mult)
            nc.vector.tensor_tensor(A3, A3, sw3[:, :, 0:47], Alu.subtract)
            nc.vector.tensor_tensor(A3, A3, rw3[:, :, 1:48], Alu.mult)
            nc.vector.tensor_reduce(accs["p1"][:], A3, AX.XY, Alu.add)
            t2 = pool.tile([P, NBLK * 48], dt.float32, tag="t2d")
            nc.vector.tensor_tensor(t2[:], rw_sh[:], rw_sh[:], Alu.mult)
            nc.vector.tensor_tensor(t2[:], t2[:], ds[:], Alu.mult)
            nc.vector.tensor_reduce(accs["p2"][:], _blk(t2[:], 48), AX.XY, Alu.add)

        # ---------- inter loss (levels emitted concurrently) ----------
        inter_lvls = [l for l in (0, 1) if f"l{l}" in parts]
        if not inter_lvls:
            nc.vector.memset(accs["inter"][:], 0.0)
        lvl_pools = {l: ctx.enter_context(tc.tile_pool(name=f"lvl{l}", bufs=1))
                     for l in inter_lvls}
        for lvl in inter_lvls:
            eng = {}
            _emit_level(nc, tc, lvl_pools[lvl], lvl, s_sh, radios[lvl],
                        aps[f"ps{lvl}"], aps[f"pw{lvl}"],
                        accs["inter" if lvl == 0 else "inter1"], aps, **eng)

        # ---------- rgb ----------
        with tc.tile_pool(name="rgb", bufs=1) as pool:
            pdt = pool.tile([P, NBLK * 3], dt.float32, tag="pdt")
            gtt = pool.tile([P, NBLK * 3], dt.float32, tag="gtt")
            nc.sync.dma_start(_blk(pdt[:], 3),
                              aps["pd"].rearrange("(b p) c -> p b c", p=P))
            nc.sync.dma_start(_blk(gtt[:], 3),
                              aps["gt"].rearrange("(b p) c -> p b c", p=P))
            d = pool.tile([P, NBLK * 3], dt.float32, tag="rgbd")
            nc.vector.tensor_tensor(d[:], pdt[:], gtt[:], Alu.subtract)
            nc.vector.tensor_tensor(d[:], d[:], d[:], Alu.mult)
            nc.vector.tensor_reduce(accs["rgb"][:], d[:], AX.X, Alu.add)

        # ---------- hash ----------
        for lvl in ((0, 1) if "hash" in parts else ()):
            with tc.tile_pool(name=f"hash{lvl}", bufs=1) as pool:
                idx = pool.tile([P, HCOLS], dt.int32, tag="hidx")
                src = aps[f"hi{lvl}"]
                nc.sync.dma_start(idx[:], bass.AP(tensor=src.tensor,
                                                  offset=src.offset,
                                                  ap=[[HROW, P], [1, HCOLS]]))
                emb = pool.tile([P, HCOLS * 2], dt.float32, tag="hemb")
                esrc = aps[f"he{lvl}"]
                nc.sync.dma_start(emb[:], bass.AP(tensor=esrc.tensor,
                                                  offset=esrc.offset,
                                                  ap=[[HROW * 2, P], [1, HCOLS * 2]]))
                sq = pool.tile([P, HCOLS * 2], dt.float32, tag="hsq")
                nc.vector.tensor_tensor(sq[:], emb[:], emb[:], Alu.mult)
                wv = pool.tile([P, HCOLS], dt.float32, tag="hw")
                sq3 = sq[:].rearrange("p (n two) -> p n two", two=2)
                nc.vector.tensor_tensor(wv[:], sq3[:, :, 0], sq3[:, :, 1], Alu.add)
                eq = pool.tile([P, HCOLS], dt.float32, tag="heq")
                nc.vector.memset(eq[:, 0:1], 0.0)
                nc.vector.tensor_tensor(eq[:, 1:HCOLS], idx[:, 1:HCOLS],
                                        idx[:, 0:HCOLS - 1], Alu.is_equal)
                S = pool.tile([P, HCOLS], dt.float32, tag="hS")
                nc.vector.tensor_tensor_scan(S[:], eq[:], wv[:], 0.0,
                                             Alu.mult, Alu.add)
                cc = pool.tile([P, HCOLS], dt.float32, tag="hcc")
                nc.vector.tensor_tensor_scan(cc[:], eq[:], ones_h[:], 0.0,
                                             Alu.mult, Alu.add)
                ratio = pool.tile([P, HCOLS], dt.float32, tag="hr")
                nc.vector.reciprocal(cc[:], cc[:])
                nc.vector.tensor_tensor(ratio[:], S[:], cc[:], Alu.mult)
                me = pool.tile([P, HCOLS], dt.float32, tag="hme")
                nc.vector.tensor_scalar(me[:, 0:HCOLS - 1], eq[:, 1:HCOLS], -1.0,
                                        1.0, Alu.mult, Alu.add)
                nc.vector.tensor_tensor(ratio[:, HALO:HALO + HROW],
                                        ratio[:, HALO:HALO + HROW],
                                        me[:, HALO:HALO + HROW], Alu.mult)
                part = pool.tile([P, 1], dt.float32, tag="hpart")
                nc.vector.tensor_reduce(part[:], ratio[:, HALO:HALO + HROW],
                                        AX.X, Alu.add)
                if lvl == 0:
                    nc.vector.tensor_copy(accs["hash"][:], part[:])
                else:
                    nc.vector.tensor_tensor(accs["hash"][:], accs["hash"][:],
                                            part[:], Alu.add)

        # ---------- combine + output ----------
        with tc.tile_pool(name="fin", bufs=1) as pool:
            tot = pool.tile([P, 1], dt.float32, tag="tot")
            nc.vector.tensor_scalar(tot[:], accs["rgb"][:], W_RGB / (R * 3), None,
                                    Alu.mult)
            nc.vector.scalar_tensor_tensor(tot[:], accs["inter"][:], W_INTER,
                                           tot[:], Alu.mult, Alu.add)
            nc.vector.scalar_tensor_tensor(tot[:], accs["inter1"][:], W_INTER,
                                           tot[:], Alu.mult, Alu.add)
            nc.vector.scalar_tensor_tensor(tot[:], accs["p1"][:], W_DIST * 2.0 / R,
                                           tot[:], Alu.mult, Alu.add)
            nc.vector.scalar_tensor_tensor(tot[:], accs["p2"][:],
                                           W_DIST / (3.0 * R), tot[:],
                                           Alu.mult, Alu.add)
            nc.vector.scalar_tensor_tensor(tot[:], accs["hash"][:],
                                           W_HASH / (NUM_SEGMENTS * 2.0), tot[:],
                                           Alu.mult, Alu.add)
            res = pool.tile([1, 1], dt.float32, tag="res")
            nc.gpsimd.tensor_reduce(res[:], tot[:], AX.C, Alu.add)
            nc.sync.dma_start(out_ap, res[:])


# ---------------- host side ----------------
_module_cache = {}


def _get_module():
    if "nc" not in _module_cache:
        _module_cache["nc"] = build_module()
    return _module_cache["nc"]


def shard_inputs(inputs):
    """Full inputs -> list of 8 per-core in_maps."""
    f32 = np.float32
    pd = np.ascontiguousarray(inputs["pd_rgbs"], f32)
    gt = np.ascontiguousarray(inputs["gt_rgbs"], f32)
    sd = np.ascontiguousarray(inputs["render_sdist"], f32)
    rw = np.ascontiguousarray(inputs["render_weights"], f32)
    ps0 = np.ascontiguousarray(inputs["prop_sdist_0"], f32)
    pw0 = np.ascontiguousarray(inputs["prop_weights_0"], f32)
    ps1 = np.ascontiguousarray(inputs["prop_sdist_1"], f32)
    pw1 = np.ascontiguousarray(inputs["prop_weights_1"], f32)
    hashes = {}
    for lvl in (0, 1):
        idx = np.asarray(inputs[f"enc_idx_{lvl}"]).astype(np.int32)
        emb = np.ascontiguousarray(inputs[f"enc_embds_{lvl}"], f32)
        idx_pad = np.full(M + 2 * HALO, -1, np.int32)
        idx_pad[HALO:HALO + M] = idx
        emb_pad = np.zeros((M + 2 * HALO, 2), f32)
        emb_pad[HALO:HALO + M] = emb
        hashes[lvl] = (idx_pad, emb_pad)

    consts = {}
    rep = lambda row: np.ascontiguousarray(np.tile(row, (P, 1)))
    for lvl, L in LVL.items():
        n2 = L["n2"]
        consts[f"c_iota16_l{lvl}"] = rep(np.tile(np.arange(n2, dtype=np.int16),
                                                 NBLK))
        LWc = L["EW"] + 24
        consts[f"c_iotap1_l{lvl}"] = rep(np.tile(
            np.arange(1, LWc + 1, dtype=np.float32), NBLK))
        msk = np.ones(NBLK * LWc, np.float32)
        msk[::LWc] = 0.0
        consts[f"c_mask_l{lvl}"] = rep(msk)
    m48 = np.ones(NBLK * 48, np.float32)
    m48[::48] = 0.0
    consts["c_mask48"] = rep(m48)
    consts["c_ones"] = rep(np.ones(HCOLS, np.float32))
    consts["c_zeros"] = rep(np.zeros(NBLK * 512, np.float32))
    consts["c_bigf"] = rep(np.full(NBLK * 512, BIGF, np.float32))

    in_maps = []
    for c in range(N_CORES):
        r0 = c * RPC
        lo = c * MPC
        im = {
            "pd": pd[r0:r0 + RPC], "gt": gt[r0:r0 + RPC],
            "sd": sd[r0:r0 + RPC], "rw": rw[r0:r0 + RPC],
            "ps0": ps0[r0:r0 + RPC], "pw0": pw0[r0:r0 + RPC],
            "ps1": ps1[r0:r0 + RPC], "pw1": pw1[r0:r0 + RPC],
        }
        for lvl in (0, 1):
            idx_pad, emb_pad = hashes[lvl]
            im[f"hi{lvl}"] = np.ascontiguousarray(idx_pad[lo:lo + HSLICE])
            im[f"he{lvl}"] = np.ascontiguousarray(
                emb_pad[lo:lo + HSLICE].reshape(-1))
        im.update(consts)
        in_maps.append(im)
    return in_maps


def kernel(**inputs) -> np.ndarray:
    nc = _get_module()
    in_maps = shard_inputs(inputs)
    res = run_bass_kernel_spmd(nc, in_maps, core_ids=list(range(N_CORES)))
    total = np.float64(0.0)
    for r in res.results:
        total += np.float64(r["out"][0, 0])
    return np.float32(total)



# revision 16
# speedup vs baseline: 1.3826x; 1.3826x over previous
"""Trainium2 Bass kernel for nn_Loss_dict_50646254354805 (NeRF-style loss).

Self-contained: accepts FULL inputs, shards across 8 NeuronCores (rays for
the per-ray losses, samples for the hash loss), runs one SPMD Bass module,
host-sums the 8 partial scalars.

Inter-loss: the reference's blur_step_function + sorted_interp_quad is
evaluated in a merged domain. Keys are uint16 quantized values (14-bit grid)
with 2-bit source tags, bitonic-merged at 2x DVE rate; per-slot values come
from the keys (grid error ~6e-5, validated ~1e-2 rel on the inter terms,
~0.5% on the total loss vs 2e-2 budget); the +-radio slopes are scattered
as exact f32 halves (their telescoping cancellation needs full precision).
Density/CDF reconstruction runs as masked prefix scans on the Pool engine;
conversions/relu/square run on the Activation engine; counts, positions and
compaction indices are uint16 DVE ops at 2-4x rate.
"""
import numpy as np

import concourse.bass as bass
import concourse.bass_isa as bass_isa
import concourse.mybir as mybir
import concourse.tile as tile
from concourse import bacc
from concourse.bass_utils import run_bass_kernel_spmd

dt = mybir.dt
Alu = mybir.AluOpType
AX = mybir.AxisListType
ACTF = mybir.ActivationFunctionType
P = 128

# problem constants
PULSE = (0.01, 0.005)
W_RGB, W_INTER, W_DIST, W_HASH = 1.0, 1.0, 0.01, 0.1
NUM_SEGMENTS = 65536
R, N = 4096, 48
M = R * N
N_CORES = 8
RPC = R // N_CORES            # rays per core (512)
NBLK = RPC // P               # ray tiles per core (4)
MPC = M // N_CORES            # hash samples per core (24576)
HALO = 64                     # hash run halo
HROW = MPC // P               # hash samples per partition (192)
HCOLS = HROW + HALO + 1       # loaded cols per partition (257)
HSLICE = HALO + MPC + HALO    # per-core hash slice length (24704)

# key quantization: key = trunc((v + OFF) * S4), tags in the low 2 bits
S4 = 63000.0
OFF = 0.02
PADK = 0xFFFC                 # pad key (tag 0, larger than any real key)

# per-level geometry
LVL = {0: dict(X=257, n2=512), 1: dict(X=97, n2=256)}
for _L in LVL.values():
    _L["EW"] = ((_L["X"] + 98 + 1 + 7) // 8) * 8        # 360 / 200
    _L["LW"] = _L["EW"] + 24                            # 384 / 224
    _L["NL"] = NBLK * _L["LW"]                          # 1536 / 896
    _L["SL"] = NBLK * _L["n2"]                          # 2048 / 1024
    _L["NW"] = NBLK * (_L["X"] - 1)                     # 1024 / 384
    _L["QWS"] = _L["LW"] - 98                           # 286 / 126
    _L["NQ"] = NBLK * _L["QWS"]


def _ts_int(eng, out, in0, imm1, op0, imm2=None, op1=None):
    """tensor_scalar with int32 immediates (for bitwise/compare ops)."""
    ins_ = [eng.lower_ap(in0), mybir.ImmediateValue(dtype=dt.int32, value=int(imm1))]
    kw = dict(op0=op0)
    if imm2 is not None:
        ins_.append(mybir.ImmediateValue(dtype=dt.int32, value=int(imm2)))
        kw["op1"] = op1
    return eng.add_instruction(mybir.InstTensorScalarPtr(
        name=eng.bass.get_next_instruction_name(),
        ins=ins_, outs=[eng.lower_ap(out)], **kw))


def _blk(ap, n2):
    return ap.rearrange("p (b n) -> p b n", b=NBLK)


def _merge_stages(VE, bufa, bufb, width, d_list, descending=False):
    """Full bitonic merge stages (ping-pong) over [P, G*width] u16 tiles."""
    cur, nxt = bufa, bufb
    for d in d_list:
        c3 = cur[:].rearrange("p (c td) -> p c td", td=2 * d)
        n3 = nxt[:].rearrange("p (c td) -> p c td", td=2 * d)
        lo_in, hi_in = c3[:, :, 0:d], c3[:, :, d:2 * d]
        if descending:
            VE.tensor_tensor(n3[:, :, 0:d], lo_in, hi_in, Alu.max)
            VE.tensor_tensor(n3[:, :, d:2 * d], lo_in, hi_in, Alu.min)
        else:
            VE.tensor_tensor(n3[:, :, 0:d], lo_in, hi_in, Alu.min)
            VE.tensor_tensor(n3[:, :, d:2 * d], lo_in, hi_in, Alu.max)
        cur, nxt = nxt, cur
    return cur, nxt


def _emit_level(nc, tc, pool, lvl, s_sh, radio, b1t, aps, accs):
    """Inter-loss pipeline for one prop level, u16 merged-domain."""
    VE, PL, ACT, SP = nc.vector, nc.gpsimd, nc.scalar, nc.sync
    L = LVL[lvl]
    X, n2, EW, LW, NL, SL, NW, QWS, NQ = (L["X"], L["n2"], L["EW"], L["LW"],
                                          L["NL"], L["SL"], L["NW"], L["QWS"],
                                          L["NQ"])
    pw = PULSE[lvl]

    def blkL(ap):
        return ap.rearrange("p (b n) -> p b n", b=NBLK)

    # ---------- per-level iota constants (DMA on the idle SP engine) ------
    iotaP1 = pool.tile([P, NL], dt.int16, tag="iotaP1")
    SP.dma_start(iotaP1[:], aps[f"c_iotap1_l{lvl}"][:, 0:NL])
    iotaC = pool.tile([P, NL], dt.int16, tag="iotaC")
    SP.dma_start(iotaC[:], aps[f"c_iotac_l{lvl}"][:, 0:NL])
    mask16 = pool.tile([P, NL], dt.uint16, tag="mask16")
    PL.memset(mask16[:], 1)
    PL.memset(blkL(mask16[:])[:, :, 0:1], 0)
    maskf = pool.tile([P, NL], dt.float32, tag="maskf")
    PL.memset(maskf[:], 1.0)
    PL.memset(blkL(maskf[:])[:, :, 0:1], 0.0)

    # ---------- inputs ----------
    xt = pool.tile([P, NBLK * X], dt.float32, tag="xt")
    SP.dma_start(_blk(xt[:], X), aps[f"ps{lvl}"].rearrange("(b p) x -> p b x", p=P))
    pwt = pool.tile([P, NW], dt.float32, tag="pwt")
    SP.dma_start(_blk(pwt[:], X - 1),
                 aps[f"pw{lvl}"].rearrange("(b p) x -> p b x", p=P))
    dinv = pool.tile([P, NW], dt.float32, tag="dinv")
    ACT.activation(dinv[:], pwt[:], ACTF.Copy, bias=1e-5)
    VE.reciprocal(dinv[:], dinv[:])

    # ---------- quantized keys ----------
    kq = pool.tile([P, NBLK * X], dt.uint16, tag="kq")
    ACT.activation(kq[:], xt[:], ACTF.Copy, scale=S4, bias=OFF * S4)
    _ts_int(VE, kq[:], kq[:], 0xFFFC, Alu.bitwise_and)

    # ---------- big merge: queries + events (from b1t), ascending ----------
    B0 = pool.tile([P, SL], dt.uint16, tag="big0")
    B1 = pool.tile([P, SL], dt.uint16, tag="big1")
    b03 = _blk(B0[:], n2)
    # pad band between queries and the event tail
    PL.memset(b03[:, :, X:n2 - 128], PADK)
    VE.tensor_copy(b03[:, :, 0:X], _blk(kq[:], X))
    # b1t holds [98 events asc | 30 pads] per (lvl,blk) group; reversed copy
    # gives [pads | events desc] as the descending tail of the bitonic input.
    b1f = b1t[:].rearrange("p (g n) -> p g n", n=128)
    g0 = lvl * NBLK
    VE.tensor_copy(b03[:, :, n2 - 128:n2], b1f[:, g0:g0 + NBLK, ::-1])
    # first stage: only the trailing 98 pairs touch real data
    d0 = n2 // 2
    VE.tensor_tensor(_blk(B1[:], n2)[:, :, d0 - 98:d0],
                     b03[:, :, d0 - 98:d0], b03[:, :, n2 - 98:n2], Alu.min)
    VE.tensor_tensor(_blk(B1[:], n2)[:, :, n2 - 98:n2],
                     b03[:, :, d0 - 98:d0], b03[:, :, n2 - 98:n2], Alu.max)
    VE.tensor_copy(_blk(B1[:], n2)[:, :, 0:d0 - 98], b03[:, :, 0:d0 - 98])
    VE.tensor_copy(_blk(B1[:], n2)[:, :, d0:n2 - 98], b03[:, :, d0:n2 - 98])
    ds_rest = [n2 // 4]
    while ds_rest[-1] > 1:
        ds_rest.append(ds_rest[-1] // 2)
    Kt, Ksc = _merge_stages(VE, B1, B0, n2, ds_rest)
    mS = _blk(Kt[:], n2)[:, :, 0:LW]       # merged keys, strided [P,NBLK,LW]

    # ---------- flags / counts (u16) ----------
    ev16 = pool.tile([P, NL], dt.uint16, tag="ev16")
    _ts_int(VE, blkL(ev16[:]), mS, 1, Alu.bitwise_and)
    em16 = pool.tile([P, NL], dt.uint16, tag="em16")
    _ts_int(VE, blkL(em16[:]), mS, 3, Alu.bitwise_and, 1, Alu.is_equal)
    ep16 = pool.tile([P, NL], dt.uint16, tag="ep16")
    _ts_int(VE, blkL(ep16[:]), mS, 3, Alu.bitwise_and, 3, Alu.is_equal)
    C16 = pool.tile([P, NL], dt.uint16, tag="C16")
    PL.tensor_tensor_scan(C16[:], mask16[:], ev16[:], 0.0, Alu.mult, Alu.add)
    Cm16 = pool.tile([P, NL], dt.uint16, tag="Cm16")
    PL.tensor_tensor_scan(Cm16[:], mask16[:], em16[:], 0.0, Alu.mult, Alu.add)

    # ---------- event position scatters ----------
    tmp16 = pool.tile([P, NL], dt.uint16, tag="tmp16")
    idx16 = pool.tile([P, NL], dt.int16, tag="idx16")
    t3 = blkL(tmp16[:])
    i3 = blkL(idx16[:])
    C3, Cm3, em3, ep3 = (blkL(C16[:]), blkL(Cm16[:]), blkL(em16[:]),
                         blkL(ep16[:]))
    pos_m = pool.tile([P, NBLK * 64], dt.uint16, tag="pos_m")
    pos_p = pool.tile([P, NBLK * 64], dt.uint16, tag="pos_p")
    for which, pos in ((0, pos_m), (1, pos_p)):
        if which == 0:
            VE.tensor_tensor(t3[:, :, 0:EW], Cm3[:, :, 0:EW], em3[:, :, 0:EW],
                             Alu.mult)
        else:
            VE.tensor_tensor(t3[:, :, 0:EW], C3[:, :, 0:EW], Cm3[:, :, 0:EW],
                             Alu.subtract)
            VE.tensor_tensor(t3[:, :, 0:EW], t3[:, :, 0:EW], ep3[:, :, 0:EW],
                             Alu.mult)
        _ts_int(VE, i3[:, :, 0:EW], t3[:, :, 0:EW], -1, Alu.add)
        for b in range(NBLK):
            PL.local_scatter(pos[:, b * 64:(b + 1) * 64],
                             iotaP1[:, b * LW:b * LW + EW].bitcast(dt.uint16),
                             idx16[:, b * LW:b * LW + EW], channels=P,
                             num_elems=64, num_idxs=EW)

    # ---------- radio scatter (exact f32 halves) ----------
    tgt16 = pool.tile([P, NBLK * 128], dt.int16, tag="tgt16")
    tg3 = _blk(tgt16[:], 128)
    pm3 = _blk(pos_m[:], 64)
    pp3 = _blk(pos_p[:], 64)
    for b in range(NBLK):
        _ts_int(VE, tg3[:, b, 0:49], pm3[:, b, 0:49], b * LW - 1, Alu.add)
        _ts_int(VE, tg3[:, b, 49:98], pp3[:, b, 0:49], b * LW - 1, Alu.add)
    PL.memset(tg3[:, :, 98:128], -1)

    radcat = pool.tile([P, NBLK * 128], dt.float32, tag="radcat")
    r3 = _blk(radcat[:], 128)
    VE.tensor_copy(r3[:, :, 0:49], _blk(radio[:], 49))
    VE.tensor_scalar(r3[:, :, 49:98], _blk(radio[:], 49), -1.0, None, Alu.mult)
    PL.memset(r3[:, :, 98:128], 0.0)
    rc_u = radcat[:].bitcast(dt.uint16).rearrange("p (n two) -> p n two", two=2)
    rad_lo = pool.tile([P, NBLK * 128], dt.uint16, tag="rad_lo")
    rad_hi = pool.tile([P, NBLK * 128], dt.uint16, tag="rad_hi")
    VE.tensor_copy(rad_lo[:], rc_u[:, :, 0])
    VE.tensor_copy(rad_hi[:], rc_u[:, :, 1])
    rl_t = pool.tile([P, NL], dt.uint16, tag="rl_t")
    rh_t = pool.tile([P, NL], dt.uint16, tag="rh_t")
    PL.local_scatter(rl_t[:], rad_lo[:], tgt16[:], channels=P,
                     num_elems=NL, num_idxs=NBLK * 128)
    PL.local_scatter(rh_t[:], rad_hi[:], tgt16[:], channels=P,
                     num_elems=NL, num_idxs=NBLK * 128)
    radio_m = pool.tile([P, NL], dt.float32, tag="radio_m")
    rm_u = radio_m[:].bitcast(dt.uint16).rearrange("p (n two) -> p n two", two=2)
    VE.tensor_copy(rm_u[:, :, 0], rl_t[:])
    VE.tensor_copy(rm_u[:, :, 1], rh_t[:])

    # ---------- dv from integer key diffs ----------
    kd = tmp16                           # tmp16 dead after pos idx
    kd3 = blkL(kd[:])
    VE.tensor_tensor(kd3[:, :, 1:EW], mS[:, :, 1:EW], mS[:, :, 0:EW - 1],
                     Alu.subtract)
    dv = pool.tile([P, NL], dt.float32, tag="dv")
    dv3 = blkL(dv[:])
    ACT.activation(dv3[:, :, 1:EW], kd3[:, :, 1:EW], ACTF.Copy, scale=1.0 / S4)

    # ---------- density chain (f32; scans on Pool) ----------
    g = pool.tile([P, NL], dt.float32, tag="g")
    PL.tensor_tensor_scan(g[:], maskf[:], radio_m[:], 0.0, Alu.mult, Alu.add)
    wg = radio_m                          # radio_m dead after g scan
    wg3 = blkL(wg[:])
    PL.memset(wg3[:, :, 0:1], 0.0)
    PL.memset(wg3[:, :, EW:LW], 0.0)
    VE.tensor_tensor(wg3[:, :, 1:EW], dv3[:, :, 1:EW], blkL(g[:])[:, :, 0:EW - 1],
                     Alu.mult)
    w = pool.tile([P, NL], dt.float32, tag="w")
    PL.tensor_tensor_scan(w[:], maskf[:], wg[:], 0.0, Alu.mult, Alu.add)
    wc = g                                # g dead after wg
    ACT.activation(wc[:], w[:], ACTF.Relu)
    scr = w                               # w dead after wc
    scr3 = blkL(scr[:])
    wc3 = blkL(wc[:])
    VE.tensor_tensor(scr3[:, :, 1:EW], wc3[:, :, 1:EW], wc3[:, :, 0:EW - 1],
                     Alu.add)
    area = wg                             # wg dead after w scan
    a3 = blkL(area[:])
    PL.memset(a3[:, :, 0:1], 0.0)
    PL.memset(a3[:, :, EW:LW], 0.0)
    # 0.5 of the trapezoid is pre-folded into the radio scale (1/(4*pw))
    VE.tensor_tensor(a3[:, :, 1:EW], scr3[:, :, 1:EW], dv3[:, :, 1:EW], Alu.mult)
    cdf = dv                              # dv dead after area
    PL.tensor_tensor_scan(cdf[:], maskf[:], area[:], 0.0, Alu.mult, Alu.add)

    # ---------- compact cdf at query slots ----------
    qf16 = em16                           # em16 dead
    _ts_int(VE, blkL(qf16[:]), mS, 3, Alu.bitwise_and, 0, Alu.is_equal)
    tq = ep16                             # ep16 dead
    VE.tensor_tensor(tq[:], iotaC[:].bitcast(dt.uint16), C16[:], Alu.subtract)
    VE.tensor_tensor(tq[:], tq[:], qf16[:], Alu.mult)
    _ts_int(VE, idx16[:], tq[:], -1, Alu.add)
    cdf_lo = ev16                         # ev16 dead
    cdf_hi = tq                           # tq value consumed by idx16
    cdf_u = cdf[:].bitcast(dt.uint16).rearrange("p (n two) -> p n two", two=2)
    VE.tensor_copy(cdf_lo[:], cdf_u[:, :, 0])
    VE.tensor_copy(cdf_hi[:], cdf_u[:, :, 1])
    cq_lo = rl_t                          # rl_t dead after radio recombine
    cq_hi = rh_t
    PL.local_scatter(cq_lo[:, 0:NQ], cdf_lo[:], idx16[:], channels=P,
                     num_elems=NQ, num_idxs=NL)
    PL.local_scatter(cq_hi[:, 0:NQ], cdf_hi[:], idx16[:], channels=P,
                     num_elems=NQ, num_idxs=NL)
    cdfq = pool.tile([P, NBLK * X], dt.float32, tag="cdfq")
    cq_u = cdfq[:].bitcast(dt.uint16).rearrange("p (b n two) -> p b n two",
                                                b=NBLK, two=2)
    VE.tensor_copy(cq_u[:, :, 0:X, 0], _blk(cq_lo[:, 0:NQ], QWS)[:, :, 0:X])
    VE.tensor_copy(cq_u[:, :, 0:X, 1], _blk(cq_hi[:, 0:NQ], QWS)[:, :, 0:X])

    # ---------- loss tail ----------
    ws = cdf[:][:, 0:NW]                  # cdf dead after split
    cqf = _blk(cdfq[:], X)
    ws3 = ws.rearrange("p (b n) -> p b n", b=NBLK)
    VE.tensor_tensor(ws3, cqf[:, :, 1:X], cqf[:, :, 0:X - 1], Alu.subtract)
    t = wc[:][:, 0:NW]                    # wc dead after scr
    VE.tensor_tensor(t, ws, pwt[:], Alu.subtract)
    r = scr[:][:, 0:NW]                   # scr dead after area
    ACT.activation(r, t, ACTF.Relu)
    sq = area[:][:, 0:NW]                 # area dead after cdf scan
    ACT.activation(sq, r, ACTF.Square)
    ttro = Ksc[:].bitcast(dt.float32)[:, 0:NW]   # merge scratch, long dead
    VE.tensor_tensor_reduce(ttro, sq, dinv[:], 1.0 / (R * (X - 1)), 0.0,
                            Alu.mult, Alu.add,
                            accs["inter" if lvl == 0 else "inter1"][:])


def build_module():
    nc = bacc.Bacc("TRN2", target_bir_lowering=False, debug=False,
                   enable_asserts=False, num_devices=N_CORES)
    aps = {}

    def din(name, shape, dtype=dt.float32):
        aps[name] = nc.dram_tensor(name, shape, dtype, kind="ExternalInput").ap()
    din("pd", [RPC, 3]); din("gt", [RPC, 3])
    din("sd", [RPC, 49]); din("rw", [RPC, 48])
    din("ps0", [RPC, 257]); din("pw0", [RPC, 256])
    din("ps1", [RPC, 97]); din("pw1", [RPC, 96])
    din("hi0", [HSLICE], dt.uint16); din("he0", [HSLICE * 2])
    din("hi1", [HSLICE], dt.uint16); din("he1", [HSLICE * 2])
    for lvl in (0, 1):
        nl = LVL[lvl]["NL"]
        din(f"c_iotap1_l{lvl}", [P, nl], dt.int16)
        din(f"c_iotac_l{lvl}", [P, nl], dt.int16)
    out_ap = nc.dram_tensor("out", [1, 1], dt.float32, kind="ExternalOutput").ap()

    with tile.TileContext(nc) as tc:
        _emit(nc, tc, aps, out_ap)
    nc.compile()
    return nc


def _emit(nc, tc, aps, out_ap):
    import contextlib
    VE, PL, ACT, SP = nc.vector, nc.gpsimd, nc.scalar, nc.sync
    with contextlib.ExitStack() as ctx:
        cpool = ctx.enter_context(tc.tile_pool(name="consts", bufs=1))
        accs = {}
        for name in ("rgb", "inter", "inter1", "p1", "p2", "hash", "hash1"):
            a = cpool.tile([P, 1], dt.float32, tag=f"acc_{name}")
            accs[name] = a

        # ---------- hash loss (emitted first: fills the DMA warmup gap) ----
        ones_h = cpool.tile([P, HCOLS], dt.float32, tag="ones_h")
        PL.memset(ones_h[:], 1.0)
        for lvl in (0, 1):
            with tc.tile_pool(name=f"hash{lvl}", bufs=1) as pool:
                idx = pool.tile([P, HCOLS], dt.uint16, tag="hidx")
                src = aps[f"hi{lvl}"]
                SP.dma_start(idx[:], bass.AP(tensor=src.tensor,
                                             offset=src.offset,
                                             ap=[[HROW, P], [1, HCOLS]]))
                emb = pool.tile([P, HCOLS * 2], dt.float32, tag="hemb")
                esrc = aps[f"he{lvl}"]
                SP.dma_start(emb[:], bass.AP(tensor=esrc.tensor,
                                             offset=esrc.offset,
                                             ap=[[HROW * 2, P], [1, HCOLS * 2]]))
                sq = pool.tile([P, HCOLS * 2], dt.float32, tag="hsq")
                ACT.activation(sq[:], emb[:], ACTF.Square)
                wv = pool.tile([P, HCOLS], dt.float32, tag="hw")
                sq3 = sq[:].rearrange("p (n two) -> p n two", two=2)
                VE.tensor_tensor(wv[:], sq3[:, :, 0], sq3[:, :, 1], Alu.add)
                eq = pool.tile([P, HCOLS], dt.float32, tag="heq")
                PL.memset(eq[:, 0:1], 0.0)
                VE.tensor_tensor(eq[:, 1:HCOLS], idx[:, 1:HCOLS],
                                 idx[:, 0:HCOLS - 1], Alu.is_equal)
                S = pool.tile([P, HCOLS], dt.float32, tag="hS")
                PL.tensor_tensor_scan(S[:], eq[:], wv[:], 0.0, Alu.mult, Alu.add)
                cc = pool.tile([P, HCOLS], dt.float32, tag="hcc")
                PL.tensor_tensor_scan(cc[:], eq[:], ones_h[:], 0.0,
                                      Alu.mult, Alu.add)
                cci = pool.tile([P, HCOLS], dt.float32, tag="hcci")
                VE.reciprocal(cci[:], cc[:])
                ratio = pool.tile([P, HCOLS], dt.float32, tag="hr")
                VE.tensor_tensor(ratio[:], S[:], cci[:], Alu.mult)
                me = pool.tile([P, HCOLS], dt.float32, tag="hme")
                VE.tensor_scalar(me[:, 0:HCOLS - 1], eq[:, 1:HCOLS], -1.0, 1.0,
                                 Alu.mult, Alu.add)
                ttro = pool.tile([P, HROW], dt.float32, tag="httro")
                VE.tensor_tensor_reduce(ttro[:], ratio[:, HALO:HALO + HROW],
                                        me[:, HALO:HALO + HROW], 1.0, 0.0,
                                        Alu.mult, Alu.add,
                                        accs["hash" if lvl == 0 else "hash1"][:])

        # ---------- shared render tables + radio + dist ----------
        spool = ctx.enter_context(tc.tile_pool(name="shared", bufs=1))
        s_sh = spool.tile([P, NBLK * 49], dt.float32, tag="s_sh")
        SP.dma_start(_blk(s_sh[:], 49), aps["sd"].rearrange("(b p) x -> p b x", p=P))
        radios = {0: spool.tile([P, NBLK * 49], dt.float32, tag="radio0",
                                name="radio0"),
                  1: spool.tile([P, NBLK * 49], dt.float32, tag="radio1",
                                name="radio1")}
        b1t = spool.tile([P, 2 * NBLK * 128], dt.uint16, tag="b1t")

        with tc.tile_pool(name="setup", bufs=1) as pool:
            rw_sh = pool.tile([P, NBLK * 48], dt.float32, tag="rw_sh")
            SP.dma_start(_blk(rw_sh[:], 48),
                         aps["rw"].rearrange("(b p) x -> p b x", p=P))
            s3 = _blk(s_sh[:], 49)
            ds = pool.tile([P, NBLK * 48], dt.float32, tag="ds")
            VE.tensor_tensor(_blk(ds[:], 48), s3[:, :, 1:49], s3[:, :, 0:48],
                             Alu.subtract)
            dsi = pool.tile([P, NBLK * 48], dt.float32, tag="dsi")
            ACT.activation(dsi[:], ds[:], ACTF.Copy, bias=1e-8)
            VE.reciprocal(dsi[:], dsi[:])
            wnorm = pool.tile([P, NBLK * 48], dt.float32, tag="wnorm")
            VE.tensor_tensor(wnorm[:], rw_sh[:], dsi[:], Alu.mult)
            wnp = pool.tile([P, NBLK * 50], dt.float32, tag="wnp")
            PL.memset(wnp[:], 0.0)
            VE.tensor_copy(_blk(wnp[:], 50)[:, :, 1:49], _blk(wnorm[:], 48))
            diff = pool.tile([P, NBLK * 49], dt.float32, tag="diff")
            wnp3 = _blk(wnp[:], 50)
            VE.tensor_tensor(_blk(diff[:], 49), wnp3[:, :, 1:50],
                             wnp3[:, :, 0:49], Alu.subtract)
            for lvl in (0, 1):
                # 1/(4*pw): includes the 0.5 of the trapezoid area
                VE.tensor_scalar(radios[lvl][:], diff[:], 1.0 / (4 * PULSE[lvl]),
                                 None, Alu.mult)

            # ---------- shared event merge (both levels, 128-wide asc) -----
            # build into b1b, partial d=64 stage into b1t, then six full
            # stages ping-pong back into b1t.
            b1b = pool.tile([P, 2 * NBLK * 128], dt.uint16, tag="b1b")
            b1g = b1b[:].rearrange("p (g n) -> p g n", n=128)
            PL.memset(b1g[:, :, 49:79], PADK)
            for lvl in (0, 1):
                pw = PULSE[lvl]
                kem = pool.tile([P, NBLK * 49], dt.uint16, tag=f"kem{lvl}")
                ACT.activation(kem[:], s_sh[:], ACTF.Copy, scale=S4,
                               bias=(OFF - pw) * S4)
                _ts_int(VE, kem[:], kem[:], 0xFFFC, Alu.bitwise_and, 1,
                        Alu.bitwise_or)
                kep = pool.tile([P, NBLK * 49], dt.uint16, tag=f"kep{lvl}")
                ACT.activation(kep[:], s_sh[:], ACTF.Copy, scale=S4,
                               bias=(OFF + pw) * S4)
                _ts_int(VE, kep[:], kep[:], 0xFFFC, Alu.bitwise_and, 3,
                        Alu.bitwise_or)
                g0 = lvl * NBLK
                VE.tensor_copy(b1g[:, g0:g0 + NBLK, 0:49], _blk(kem[:], 49))
                VE.tensor_copy(b1g[:, g0:g0 + NBLK, 79:128],
                               _blk(kep[:], 49)[:, :, ::-1])
            # partial first stage (d=64): only pairs (15..63, 79..127) matter
            b1n = b1t[:].rearrange("p (g n) -> p g n", n=128)
            VE.tensor_tensor(b1n[:, :, 15:64], b1g[:, :, 15:64],
                             b1g[:, :, 79:128], Alu.min)
            VE.tensor_tensor(b1n[:, :, 79:128], b1g[:, :, 15:64],
                             b1g[:, :, 79:128], Alu.max)
            VE.tensor_copy(b1n[:, :, 0:15], b1g[:, :, 0:15])
            VE.tensor_copy(b1n[:, :, 64:79], b1g[:, :, 64:79])
            res, _ = _merge_stages(VE, b1t, b1b, 128, [32, 16, 8, 4, 2, 1])
            assert res is b1t

            # ---------- distortion ----------
            mask48 = pool.tile([P, NBLK * 48], dt.float32, tag="mask48")
            PL.memset(mask48[:], 1.0)
            PL.memset(_blk(mask48[:], 48)[:, :, 0:1], 0.0)
            mid = pool.tile([P, NBLK * 48], dt.float32, tag="mid")
            VE.tensor_tensor(_blk(mid[:], 48), s3[:, :, 1:49], s3[:, :, 0:48],
                             Alu.add)   # 2*mid; the 0.5 folds into W_DIST
            wm = pool.tile([P, NBLK * 48], dt.float32, tag="wm")
            VE.tensor_tensor(wm[:], rw_sh[:], mid[:], Alu.mult)
            Cin = pool.tile([P, NBLK * 48], dt.float32, tag="Cin")
            PL.tensor_tensor_scan(Cin[:], mask48[:], rw_sh[:], 0.0,
                                  Alu.mult, Alu.add)
            Sin = pool.tile([P, NBLK * 48], dt.float32, tag="Sin")
            PL.tensor_tensor_scan(Sin[:], mask48[:], wm[:], 0.0,
                                  Alu.mult, Alu.add)
            A = pool.tile([P, NBLK * 47], dt.float32, tag="A47")
            m3 = _blk(mid[:], 48)
            c3 = _blk(Cin[:], 48)
            sw3 = _blk(Sin[:], 48)
            rw3 = _blk(rw_sh[:], 48)
            A3 = _blk(A[:], 47)
            VE.tensor_tensor(A3, m3[:, :, 1:48], c3[:, :, 0:47], Alu.mult)
            VE.tensor_tensor(A3, A3, sw3[:, :, 0:47], Alu.subtract)
            ttro = pool.tile([P, NBLK * 47], dt.float32, tag="dttro")
            VE.tensor_tensor_reduce(_blk(ttro[:], 47), A3, rw3[:, :, 1:48],
                                    1.0, 0.0, Alu.mult, Alu.add, accs["p1"][:],
                                    opt_aps=False)
            t2 = pool.tile([P, NBLK * 48], dt.float32, tag="t2d")
            VE.tensor_tensor(t2[:], rw_sh[:], rw_sh[:], Alu.mult)
            ttro2 = pool.tile([P, NBLK * 48], dt.float32, tag="dttro2")
            VE.tensor_tensor_reduce(ttro2[:], t2[:], ds[:], 1.0, 0.0,
                                    Alu.mult, Alu.add, accs["p2"][:])

        # ---------- rgb ----------
        with tc.tile_pool(name="rgb", bufs=1) as pool:
            pdt = pool.tile([P, NBLK * 3], dt.float32, tag="pdt")
            gtt = pool.tile([P, NBLK * 3], dt.float32, tag="gtt")
            SP.dma_start(_blk(pdt[:], 3), aps["pd"].rearrange("(b p) c -> p b c", p=P))
            SP.dma_start(_blk(gtt[:], 3), aps["gt"].rearrange("(b p) c -> p b c", p=P))
            d = pool.tile([P, NBLK * 3], dt.float32, tag="rgbd")
            VE.tensor_tensor(d[:], pdt[:], gtt[:], Alu.subtract)
            dsq = pool.tile([P, NBLK * 3], dt.float32, tag="rgbsq")
            ACT.activation(dsq[:], d[:], ACTF.Square, accum_out=accs["rgb"][:])

        # ---------- inter loss (levels emitted concurrently) ----------
        lvl_pools = {l: ctx.enter_context(tc.tile_pool(name=f"lvl{l}", bufs=1))
                     for l in (0, 1)}
        for lvl in (0, 1):
            _emit_level(nc, tc, lvl_pools[lvl], lvl, s_sh, radios[lvl],
                        b1t, aps, accs)

        # ---------- combine + output ----------
        with tc.tile_pool(name="fin", bufs=1) as pool:
            tot = pool.tile([P, 1], dt.float32, tag="tot")
            VE.tensor_scalar(tot[:], accs["rgb"][:], W_RGB / (R * 3), None,
                             Alu.mult)
            VE.scalar_tensor_tensor(tot[:], accs["inter"][:], W_INTER,
                                    tot[:], Alu.mult, Alu.add)
            VE.scalar_tensor_tensor(tot[:], accs["inter1"][:], W_INTER,
                                    tot[:], Alu.mult, Alu.add)
            VE.scalar_tensor_tensor(tot[:], accs["p1"][:], W_DIST / R,
                                    tot[:], Alu.mult, Alu.add)
            VE.scalar_tensor_tensor(tot[:], accs["p2"][:], W_DIST / (3.0 * R),
                                    tot[:], Alu.mult, Alu.add)
            VE.scalar_tensor_tensor(tot[:], accs["hash"][:],
                                    W_HASH / (NUM_SEGMENTS * 2.0), tot[:],
                                    Alu.mult, Alu.add)
            VE.scalar_tensor_tensor(tot[:], accs["hash1"][:],
                                    W_HASH / (NUM_SEGMENTS * 2.0), tot[:],
                                    Alu.mult, Alu.add)
            res = pool.tile([P, 1], dt.float32, tag="res")
            PL.partition_all_reduce(res[:], tot[:], channels=P,
                                    reduce_op=bass_isa.ReduceOp.add)
            SP.dma_start(out_ap, res[0:1, 0:1])


# ---------------- host side ----------------
_module_cache = {}


def _get_module():
    if "nc" not in _module_cache:
        _module_cache["nc"] = build_module()
    return _module_cache["nc"]


def shard_inputs(inputs):
    """Full inputs -> list of 8 per-core in_maps."""
    f32 = np.float32
    pd = np.ascontiguousarray(inputs["pd_rgbs"], f32)
    gt = np.ascontiguousarray(inputs["gt_rgbs"], f32)
    sd = np.ascontiguousarray(inputs["render_sdist"], f32)
    rw = np.ascontiguousarray(inputs["render_weights"], f32)
    ps0 = np.ascontiguousarray(inputs["prop_sdist_0"], f32)
    pw0 = np.ascontiguousarray(inputs["prop_weights_0"], f32)
    ps1 = np.ascontiguousarray(inputs["prop_sdist_1"], f32)
    pw1 = np.ascontiguousarray(inputs["prop_weights_1"], f32)
    hashes = {}
    for lvl in (0, 1):
        idx = np.asarray(inputs[f"enc_idx_{lvl}"]).astype(np.int64)
        emb = np.ascontiguousarray(inputs[f"enc_embds_{lvl}"], f32)
        idx_pad = np.empty(M + 2 * HALO, np.uint16)
        idx_pad[HALO:HALO + M] = idx.astype(np.uint16)
        # pads must differ from the adjacent real idx (run-break sentinels)
        idx_pad[:HALO] = np.uint16((int(idx[0]) + 1) & 0xFFFF)
        idx_pad[HALO + M:] = np.uint16((int(idx[-1]) + 1) & 0xFFFF)
        emb_pad = np.zeros((M + 2 * HALO, 2), f32)
        emb_pad[HALO:HALO + M] = emb
        hashes[lvl] = (idx_pad, emb_pad)

    consts = {}
    for lvl, L in LVL.items():
        LW, QWS = L["LW"], L["QWS"]
        p1 = np.tile(np.arange(1, LW + 1, dtype=np.int16), NBLK)
        consts[f"c_iotap1_l{lvl}"] = np.ascontiguousarray(np.tile(p1, (P, 1)))
        ic = np.concatenate([np.arange(1, LW + 1, dtype=np.int16) + b * QWS
                             for b in range(NBLK)])
        consts[f"c_iotac_l{lvl}"] = np.ascontiguousarray(np.tile(ic, (P, 1)))

    in_maps = []
    for c in range(N_CORES):
        r0 = c * RPC
        lo = c * MPC
        im = {
            "pd": pd[r0:r0 + RPC], "gt": gt[r0:r0 + RPC],
            "sd": sd[r0:r0 + RPC], "rw": rw[r0:r0 + RPC],
            "ps0": ps0[r0:r0 + RPC], "pw0": pw0[r0:r0 + RPC],
            "ps1": ps1[r0:r0 + RPC], "pw1": pw1[r0:r0 + RPC],
        }
        for lvl in (0, 1):
            idx_pad, emb_pad = hashes[lvl]
            im[f"hi{lvl}"] = np.ascontiguousarray(idx_pad[lo:lo + HSLICE])
            im[f"he{lvl}"] = np.ascontiguousarray(
                emb_pad[lo:lo + HSLICE].reshape(-1))
        im.update(consts)
        in_maps.append(im)
    return in_maps


def kernel(**inputs) -> np.ndarray:
    nc = _get_module()
    in_maps = shard_inputs(inputs)
    res = run_bass_kernel_spmd(nc, in_maps, core_ids=list(range(N_CORES)))
    total = np.float64(0.0)
    for r in res.results:
        total += np.float64(r["out"][0, 0])
    return np.float32(total)
